# revision 26
# baseline (speedup 1.0000x reference)
"""Sliding-window attention Trainium2 Bass kernel (v5).

Problem: B=4, H=32, L=4096, D=128, window=512.
reference: attends over the LAST w=512 key/value positions; query row i may
only see window slot j when j <= i (slots are key positions L-w+j).

Sharding: B*H = 128 (b,h) pairs split across 8 cores -> 16 heads/core.
Pure data parallelism, no collectives.

v5 design (per group of 512 queries; Q/K/P/V fp16, accum f32):
  S^T chunks [128, 512] = (K^T chunk)^T . (Q^T group)     4 MM  (PE)
  pA = exp(sA/sqrt(D))          ACT exact exp             1 op  [128,1024]
  pB = schraudolph(sB/sqrt(D))  DVE fast-exp bit trick    1 op  [128,1024]
        (tensor_scalar mult+add, f32 PSUM -> int16 bits of fp16; verified
         bit-exact round-to-nearest on HW; ~2% rel err on half the slots
         -> ~5e-3 final max-rel-err, tolerance is 2e-2)
  PV transposed with fused denominator:  16 MM of N=129   (PE)
        out[q128, 0:128] += p_blk^T @ v_chunk ; out[:,128] += rowsum(p_blk)
        via ones column appended to V. N=129 MMs with per-MM weight swap
        sustain 57ns (measured) -- LDWEIGHTS hides in the reorder window.
  O copy PSUM->SBUF split: DVE takes subtiles 0,1; ACT takes 2,3
  DMA out [128, 516] f32 (128 d-cols + denominator col per subtile)
  Final softmax divide happens on host at unshard time.

No row-sum matmuls, no partial-sum adds, no reciprocal, no broadcast.
Engine budget/group: PE 1824ns (wall), ACT ~1630ns, DVE ~1660ns.
PSUM: S 2x[128,1024] ring2 (4 banks) + O [128,1024] ring2 (4 banks) = 8.
g==0 masking: S/PV skip chunks a query block cannot see (c<=jj), diagonal
chunk masked by a triangle multiply on P; denominator stays exact because
the ones column only accumulates over the MMs actually issued.
"""

import math
from contextlib import ExitStack

import numpy as np

N_CORES = 8
B, H, L, D = 4, 32, 4096, 128
W = 512            # window
HEADS_PER_CORE = (B * H) // N_CORES   # 16
QG = 512           # queries per group
NG = L // QG       # groups per head (8)
NCHUNK = W // 128  # 4 window chunks
SCALE = 1.0 / math.sqrt(D)

# Schraudolph fp16 fast-exp: bits = round(A*s + B), bits viewed as fp16.
# A folds in the 1/sqrt(D) softmax scale; B is mean-centered so the
# approximate half carries no systematic weight vs the exact half.
LN2 = 0.6931471805599453
SCHR_A = 1024.0 / LN2 * SCALE
SCHR_B = 15360.0 - 58.92

VCOLS = 132        # v chunk tile width (129 used, padded for alignment)
OCOLS = 516        # out tile: 4 subtiles x (128 d + 1 denom)

_COMPILED = None


def _build():
    import concourse.tile as tile
    from concourse import bacc, mybir

    nc = bacc.Bacc("TRN2", target_bir_lowering=False, debug=False,
                   num_devices=N_CORES)

    f16 = mybir.dt.float16
    f32 = mybir.dt.float32
    i16 = mybir.dt.int16

    qT = nc.dram_tensor("qT", [HEADS_PER_CORE, D, L], f16, kind="ExternalInput").ap()
    kT = nc.dram_tensor("kT", [HEADS_PER_CORE, D, W], f16, kind="ExternalInput").ap()
    vA = nc.dram_tensor("vA", [HEADS_PER_CORE, NCHUNK, 128, VCOLS], f16,
                        kind="ExternalInput").ap()
    tri = nc.dram_tensor("tri", [128, 128], f16, kind="ExternalInput").ap()
    oU = nc.dram_tensor("oU", [HEADS_PER_CORE, NG, 128, OCOLS], f32,
                        kind="ExternalOutput").ap()

    with tile.TileContext(nc) as tc:
        with ExitStack() as ctx:
            const = ctx.enter_context(tc.tile_pool(name="const", bufs=1))
            kt_pool = ctx.enter_context(tc.tile_pool(name="kt", bufs=3))
            v_pool = ctx.enter_context(tc.tile_pool(name="v", bufs=3))
            q_pool = ctx.enter_context(tc.tile_pool(name="q", bufs=2 * NG))
            p_pool = ctx.enter_context(tc.tile_pool(name="p", bufs=4))
            out_pool = ctx.enter_context(tc.tile_pool(name="out", bufs=4))
            s_psum = ctx.enter_context(tc.tile_pool(name="s_ps", bufs=2, space="PSUM"))
            o_psum = ctx.enter_context(tc.tile_pool(name="o_ps", bufs=2, space="PSUM"))

            # prefire the ACT exp table load on a scratch tile
            warm_t = const.tile([1, 2], f32, tag="warm")
            nc.gpsimd.memset(warm_t[:], 0)
            nc.scalar.activation(warm_t[:, 0:1], warm_t[:, 1:2],
                                 mybir.ActivationFunctionType.Exp)

            tri_t = const.tile([128, 128], f16, tag="tri")
            nc.gpsimd.dma_start(tri_t[:], tri[:])

            head_tiles = {}

            def load_kq0(h):
                """kt + first q tile -- just enough for front(h, 0)."""
                kt_t = kt_pool.tile([128, W], f16, tag="kt")
                nc.sync.dma_start(kt_t[:], kT[h])
                qt0 = q_pool.tile([128, QG], f16, tag="q")
                nc.sync.dma_start(qt0[:], qT[h, :, 0:QG])
                head_tiles[h] = [kt_t, None, [qt0]]

            def load_v(h, c0, c1):
                ht = head_tiles[h]
                if ht[1] is None:
                    ht[1] = v_pool.tile([128, NCHUNK * VCOLS], f16, tag="v",
                                        name="v_t")
                for c in range(c0, c1):
                    nc.sync.dma_start(ht[1][:, c * VCOLS:(c + 1) * VCOLS],
                                      vA[h, c])

            def load_q(h, i0, i1):
                ht = head_tiles[h]
                for i in range(i0, min(i1, NG)):
                    qt_t = q_pool.tile([128, QG], f16, tag="q", name="qt_t")
                    nc.sync.dma_start(qt_t[:], qT[h, :, i * QG:(i + 1) * QG])
                    ht[2].append(qt_t)

            def emit_loads(h, g):
                """<=2 load DMAs per iteration for head h+1 (plus the head-0
                bootstrap), one group earlier than strictly needed."""
                if h == 0:
                    if g == 0:
                        load_v(0, 0, 4)
                        load_q(0, 1, 2)
                    elif g == 1:
                        load_q(0, 2, 4)
                    elif g == 2:
                        load_q(0, 4, 6)
                    elif g == 3:
                        load_q(0, 6, 8)
                nxt = h + 1
                if nxt >= HEADS_PER_CORE:
                    return
                if g == 1:
                    load_kq0(nxt)
                elif g == 2:
                    load_v(nxt, 0, 2)
                elif g == 3:
                    load_v(nxt, 2, 4)
                elif g == 4:
                    load_q(nxt, 1, 3)
                elif g == 5:
                    load_q(nxt, 3, 5)
                elif g == 6:
                    load_q(nxt, 5, 7)
                elif g == 7:
                    load_q(nxt, 7, 8)

            def emit_front(h, g):
                """S matmuls + exp (ACT half / DVE schraudolph half) +
                post-exp triangle mask for g==0."""
                kt_t, v_t, qt_tiles = head_tiles[h]
                qt_t = qt_tiles[g]
                halves = []
                for half in range(2):
                    s_ps = s_psum.tile([128, 2 * QG], f32, tag="s")
                    for ci in range(2):
                        c = half * 2 + ci
                        # g==0: queries [0, c*128) can't see chunk c -- skip.
                        q_lo = c * 128 if g == 0 else 0
                        nc.tensor.matmul(
                            s_ps[:, ci * QG + q_lo:(ci + 1) * QG],
                            lhsT=kt_t[:, c * 128:(c + 1) * 128],
                            rhs=qt_t[:, q_lo:],
                            start=True, stop=True,
                        )
                    halves.append(s_ps)
                pA_t = p_pool.tile([128, 2 * QG], f16, tag="pA")
                pB_t = p_pool.tile([128, 2 * QG], f16, tag="pB")
                nc.scalar.activation(pA_t[:], halves[0][:],
                                     mybir.ActivationFunctionType.Exp,
                                     scale=SCALE)
                # schraudolph in two per-bank ops so each S psum bank frees
                # as early as possible for the next group's S matmuls
                nc.vector.tensor_scalar(
                    pB_t[:].bitcast(i16), halves[1][:],
                    SCHR_A, SCHR_B,
                    mybir.AluOpType.mult, mybir.AluOpType.add)
                if g == 0:
                    for half in range(2):
                        p_t = pA_t if half == 0 else pB_t
                        for ci in range(2):
                            c = half * 2 + ci
                            dblk = slice(ci * QG + c * 128,
                                         ci * QG + (c + 1) * 128)
                            nc.vector.tensor_mul(p_t[:, dblk], p_t[:, dblk],
                                                 tri_t[:])
                return (h, g, pA_t, pB_t)

            def emit_back(stage):
                """Transposed PV with fused denominator (one group behind)."""
                h, g, pA_t, pB_t = stage
                v_t = head_tiles[h][1]
                o_ps = o_psum.tile([128, 2 * QG], f32, tag="ops")
                for jj in range(4):
                    cmax = jj if g == 0 else NCHUNK - 1
                    for c in range(cmax + 1):
                        p_t, ci = (pA_t, c) if c < 2 else (pB_t, c - 2)
                        nc.tensor.matmul(
                            o_ps[:, jj * 256:jj * 256 + 129],
                            lhsT=p_t[:, ci * QG + jj * 128:
                                     ci * QG + (jj + 1) * 128],
                            rhs=v_t[:, c * VCOLS:c * VCOLS + 129],
                            start=(c == 0), stop=(c == cmax),
                        )
                return o_ps

            def emit_out(stage, o_ps):
                """PSUM->SBUF copy split across DVE/ACT + store DMA."""
                h, g, pA_t, pB_t = stage
                out_t = out_pool.tile([128, OCOLS], f32, tag="o")
                src = o_ps[:].rearrange("p (s c) -> p s c", s=4)[:, :, 0:129]
                dst = out_t[:].rearrange("p (s c) -> p s c", c=129)
                nc.vector.tensor_copy(dst[:, 0:1], src[:, 0:1])
                nc.scalar.copy(dst[:, 1:4], src[:, 1:4])
                nc.sync.dma_start(oU[h, g], out_t[:])
                if g == NG - 1:
                    del head_tiles[h]

            pipe = []
            load_kq0(0)
            for it in range(HEADS_PER_CORE * NG):
                h, g = divmod(it, NG)
                cur = emit_front(h, g)
                if pipe:
                    st = pipe.pop(0)
                    o_ps = emit_back(st)
                    emit_out(st, o_ps)
                emit_loads(h, g)
                pipe.append(cur)
            st = pipe.pop(0)
            o_ps = emit_back(st)
            emit_out(st, o_ps)

    nc.compile()
    return nc


def _get_compiled():
    global _COMPILED
    if _COMPILED is None:
        _COMPILED = _build()
    return _COMPILED


def _make_in_maps(query, keys, values):
    q = np.asarray(query, dtype=np.float32)
    k = np.asarray(keys, dtype=np.float32)
    v = np.asarray(values, dtype=np.float32)

    qf = q.reshape(B * H, L, D)
    kf = k.reshape(B * H, L, D)[:, L - W:, :]
    vf = v.reshape(B * H, L, D)[:, L - W:, :]

    # within a diagonal 128x128 block: query qq sees slot jj iff qq >= jj
    tri = (np.arange(128)[None, :] >= np.arange(128)[:, None]).astype(np.float16)

    in_maps = []
    for core in range(N_CORES):
        s = slice(core * HEADS_PER_CORE, (core + 1) * HEADS_PER_CORE)
        vc = vf[s].reshape(HEADS_PER_CORE, NCHUNK, 128, D).astype(np.float16)
        va = np.zeros((HEADS_PER_CORE, NCHUNK, 128, VCOLS), dtype=np.float16)
        va[:, :, :, :D] = vc
        va[:, :, :, D] = 1.0
        in_maps.append({
            "qT": np.ascontiguousarray(qf[s].transpose(0, 2, 1)).astype(np.float16),
            "kT": np.ascontiguousarray(kf[s].transpose(0, 2, 1)).astype(np.float16),
            "vA": va,
            "tri": tri,
        })
    return in_maps


def kernel(query, keys, values, window_size):
    from concourse.bass_utils import run_bass_kernel_spmd

    w = int(window_size)
    assert np.asarray(query).shape == (B, H, L, D) and w == W

    nc = _get_compiled()
    in_maps = _make_in_maps(query, keys, values)
    res = run_bass_kernel_spmd(nc, in_maps, core_ids=list(range(N_CORES)))
    outs = []
    for c in range(N_CORES):
        o = res.results[c]["oU"]                     # [16, 8, 128, 516]
        o = o.reshape(HEADS_PER_CORE, NG, 128, 4, 129)
        num = o[..., :D]                             # [16, 8, 128q, 4j, 128d]
        den = o[..., D]
        out = num / den[..., None]
        # [h, g, q, j, d] -> [h, g, j, q, d] -> [h, L, d]
        out = out.transpose(0, 1, 3, 2, 4).reshape(HEADS_PER_CORE, L, D)
        outs.append(out)
    return np.concatenate(outs, axis=0).reshape(B, H, L, D).astype(np.float32)
